# revision 21
# baseline (speedup 1.0000x reference)
"""DTM (distance-to-measure) kernel for Trainium2, 8 NeuronCores. V3.2.

Math: for each (batch b, grid point g): with d2[m] = ||g - x_m||^2 and
bound = 0.3 * sum(w), the reference's sort+cumsum+searchsorted pipeline equals
F(t*) at the weighted-quantile threshold t* of the d2 distribution, where
  F(t) = sum(w*min(d2,t)) + t*(bound - sum(w))
is exact for ANY t (concave, maximized at t*), so t only needs ~0.1 accuracy
for ~5e-3 output error.

V3 strategy (vs V2's weighted STT bisection at 1x DVE throughput):
  - Direction decisions use UNWEIGHTED counts: w is independent of position,
    so the count-quantile matches the weighted quantile to ~1%, and the
    final F eval washes t error out quadratically. Unweighted counts run as
    plain tensor_scalar (is_lt, *k, accum) which supports the DVE 4x_2p
    perf mode (0.26 ns/col vs scalar_tensor_tensor's 1.04): all wide
    operands f16/packed/SBUF; per-partition scalars exempt.
  - Newton updates instead of bisection: count ops scale bits by k=-p/V so
    accum = -p/V*cnt; one tiny STT per tile computes
    t' = (acc - k*0.3V) + t = t + p/V*(0.3V - cnt).
    Schedule (V,p): (256,2.6),(256,1.8),(2048,3.0),(2048,2.4).
  - Finals use a blockwise-constant weight approximation: points are sorted
    by weight, so each 256-col block has nearly-constant weight c_B; then
    F ~= sum_B c_B * sum_B min(d2,t) + t*(0.3*sw - sum_B c_B*|B|), computed
    as NB tensor_scalar ops at 4x (min, *c_B, accum into accF columns) plus
    one accum-reduce. Replaces the 1x full-width weighted STT (2.1us/tile
    -> ~1.0us/tile). Host-simulated rel err: 4-8e-3 across seeds (gate 2e-2).
  - Matmuls in f16 (1 cy/row vs fp32's 4).
Units of 4/4/5 tiles run iterations count+update in sequence; all
cross-iteration scalar-port reads are >=4 ops stale (HW hazard: scalar/
in1-port reads 1-op fresh return stale data; rd0-port fresh reads are fine).
d2 tiles are double-buffered by rep parity so REPS>1 pipelines cleanly.

Device mapping per core: one batch, 1664 grid points as 13 tiles of 128
partitions; PE computes d2 tiles via K=4 augmented f16 matmul; ACT
Relu-copies PSUM->SBUF f16; DVE runs Newton counts + blockwise finals;
host does the final sqrt.
"""
import sys
sys.path.insert(0, "/opt/trn_rl_repo")

import numpy as np
import concourse.bass as bass
from concourse import mybir

M0 = 0.3
B, M, N = 2, 2048, 6561
P = 128
NT = 13              # tiles per core
NSH = NT * P         # 1664 grid points per core
NSHARDS = 4          # grid shards (x2 batches = 8 cores)
NPAD = NSH * NSHARDS # 6656
CHUNK = 512
NCH = M // CHUNK
SCHED = [(256, 2.6), (256, 1.8), (1024, 3.0), (2048, 2.4)]
NIT = len(SCHED)
T0 = 1.0
REPS = 1             # bench amplifier: whole compute pipeline repeated

UNITS = [(0, 1, 2, 3), (4, 5, 6, 7), (8, 9, 10, 11, 12)]
DEBUG_OUT = None     # None | "t" (dump final thresholds) | "msum"

_NC = None


def _build():
    global _NC
    if _NC is not None:
        return _NC
    nc = bass.Bass()
    f32 = mybir.dt.float32
    f16 = mybir.dt.float16

    gaug = nc.dram_tensor("gaug", [4, NSH], f16, kind="ExternalInput")
    xaug = nc.dram_tensor("xaug", [4, M], f16, kind="ExternalInput")
    wrow = nc.dram_tensor("wrow", [1, M], f16, kind="ExternalInput")
    bnd2 = nc.dram_tensor("bnd2", [1, 1], f32, kind="ExternalInput")
    out = nc.dram_tensor("out", [P, NT], f32, kind="ExternalOutput")

    sb_gaug = nc.alloc_sbuf_tensor("sb_gaug", [4, NSH], f16)
    sb_xaug = nc.alloc_sbuf_tensor("sb_xaug", [4, M], f16)
    sb_w = nc.alloc_sbuf_tensor("sb_w", [P, M], f16)
    sb_bnd2 = nc.alloc_sbuf_tensor("sb_bnd2", [P, 1], f32)
    sb_out = nc.alloc_sbuf_tensor("sb_out", [P, NT], f32)
    NDB = 2 if REPS > 1 else 1
    d2 = [[nc.alloc_sbuf_tensor(f"d2_{r}_{t}", [P, M], f16) for t in range(NT)]
          for r in range(NDB)]
    scratch = [nc.alloc_sbuf_tensor(f"scratch_{i}", [P, M], f16) for i in range(4)]
    NU = len(UNITS)
    MAXSL = max(len(u) for u in UNITS)
    tb = [[[nc.alloc_sbuf_tensor(f"tb_{u}_{j}_{i}", [P, 1], f32)
            for i in range(MAXSL)] for j in range(2)] for u in range(NU)]
    acc = [[nc.alloc_sbuf_tensor(f"acc_{u}_{i}", [P, 1], f32)
            for i in range(MAXSL)] for u in range(NU)]
    msum = [nc.alloc_sbuf_tensor(f"msum_{t}", [P, 1], f32) for t in range(NT)]
    sb_t0 = nc.alloc_sbuf_tensor("sb_t0", [P, 1], f32)
    ps = [nc.alloc_psum_tensor(f"ps_{i}", [P, M], f32) for i in range(2)]

    Alu = mybir.AluOpType
    Act = mybir.ActivationFunctionType

    # per-iteration constants. TSP-with-accum semantics (HW-verified):
    #   out = in0 op0 scalar1;  accum = scalar2 + reduce(out, op1)
    # so the count op yields acc = cnt - 0.3V directly (init = -0.3V), and
    # the update is t' = acc*(-p/V) + t in one small STT.
    INITS = [float(-M0 * V) for (V, p) in SCHED]
    GAINS = [float(np.float32(-p / V)) for (V, p) in SCHED]

    TFJ = (NIT - 1) % 2  # ping-pong slot holding the final t

    with (
        nc.Block() as block,
        nc.semaphore("dma_sem") as dma_sem,
        nc.semaphore("ge_sem") as ge_sem,
        nc.semaphore("mm_sem") as mm_sem,
        nc.semaphore("d2_sem") as d2_sem,
        nc.semaphore("dve_sem") as dve_sem,
    ):
        @block.sync
        def _(sync):
            sync.dma_start(out=sb_gaug[:], in_=gaug[:, :]).then_inc(ge_sem, 16)
            sync.dma_start(out=sb_xaug[:], in_=xaug[:, :]).then_inc(ge_sem, 16)
            sync.dma_start(out=sb_w[:], in_=wrow[:, :].to_broadcast((P, M))).then_inc(dma_sem, 16)
            sync.dma_start(out=sb_bnd2[:], in_=bnd2[:, :].to_broadcast((P, 1))).then_inc(dma_sem, 16)

        @block.tensor
        def _(tensor):
            tensor.wait_ge(ge_sem, 32)
            for rep in range(REPS):
                for t in range(NT):
                    seq = rep * NT + t
                    if seq >= 2:
                        tensor.wait_ge(d2_sem, seq - 1)  # ACT drained ps[seq%2]
                    mm = None
                    for c in range(NCH):
                        mm = tensor.matmul(
                            out=ps[seq % 2][:, c * CHUNK:(c + 1) * CHUNK],
                            lhsT=sb_gaug[:, t * P:(t + 1) * P],
                            rhs=sb_xaug[:, c * CHUNK:(c + 1) * CHUNK],
                            start=True, stop=True)
                    mm.then_inc(mm_sem, 1)

        @block.scalar
        def _(scalar):
            for rep in range(REPS):
                if rep >= NDB:
                    # d2 bank rewrite: DVE of rep-NDB must be done reading
                    scalar.wait_ge(dve_sem, rep - NDB + 1)
                for t in range(NT):
                    seq = rep * NT + t
                    scalar.wait_ge(mm_sem, seq + 1)
                    scalar.activation(out=d2[rep % NDB][t][:], in_=ps[seq % 2][:],
                                      func=Act.Relu).then_inc(d2_sem, 1)

        @block.vector
        def _(vector):
            vector.memset(sb_t0[:], T0)
            vector.memset(acc[0][0][:], 0.0)  # spacers: age sb_t0 before
            vector.memset(acc[0][1][:], 0.0)  # its first scalar-port read
            vector.wait_ge(dma_sem, 32)
            last = None

            def unit_counts(u, it, db):
                tiles = UNITS[u]
                V, _ = SCHED[it]
                src = [sb_t0] * len(tiles) if it == 0 else tb[u][(it + 1) % 2]
                for i, t in enumerate(tiles):
                    vector.tensor_scalar(
                        out=scratch[(2 * u + i) % 4][:, :V],
                        in0=d2[db][t][:, :V],
                        scalar1=src[i][:], scalar2=INITS[it],
                        op0=Alu.is_lt, op1=Alu.add,
                        accum_out=acc[u][i][:])

            def unit_upds(u, it):
                nsl = len(UNITS[u])
                src = [sb_t0] * nsl if it == 0 else tb[u][(it + 1) % 2]
                dst = tb[u][it % 2]
                for i in range(nsl):
                    vector.scalar_tensor_tensor(
                        out=dst[i][:], in0=acc[u][i][:], scalar=GAINS[it],
                        in1=src[i][:], op0=Alu.mult, op1=Alu.add)

            def unit_finals(u, db):
                tiles = UNITS[u]
                tf = tb[u][TFJ]
                # exact weighted final: STT is 1x but measured cheaper than
                # blockwise TSP+accum ops (accum blocks the 4x perf mode)
                for i, t in enumerate(tiles):
                    vector.scalar_tensor_tensor(
                        out=scratch[(2 * u + i) % 4][:],
                        in0=d2[db][t][:], scalar=tf[i][:],
                        in1=sb_w[:], op0=Alu.min, op1=Alu.mult,
                        accum_out=msum[t][:])

            def unit_outs(u):
                nonlocal last
                tiles = UNITS[u]
                tf = tb[u][TFJ]
                for i, t in enumerate(tiles):
                    if DEBUG_OUT == "t":
                        last = vector.tensor_copy(out=sb_out[:, t:t + 1],
                                                  in_=tf[i][:])
                    elif DEBUG_OUT == "msum":
                        last = vector.tensor_copy(out=sb_out[:, t:t + 1],
                                                  in_=msum[t][:])
                    else:
                        last = vector.scalar_tensor_tensor(
                            out=sb_out[:, t:t + 1], in0=tf[i][:],
                            scalar=sb_bnd2[:], in1=msum[t][:],
                            op0=Alu.mult, op1=Alu.add)

            for rep in range(REPS):
                db = rep % NDB
                for u in range(len(UNITS)):
                    for it in range(NIT):
                        if it == 0:
                            vector.wait_ge(d2_sem, rep * NT + max(UNITS[u]) + 1)
                        unit_counts(u, it, db)
                        unit_upds(u, it)
                    unit_finals(u, db)
                    unit_outs(u)
                last.then_inc(dve_sem, 1)

        @block.sync
        def _(sync):
            sync.wait_ge(dve_sem, REPS)
            sync.dma_start(out=out[:, :], in_=sb_out[:]).then_inc(dma_sem, 16)
            sync.wait_ge(dma_sem, 48)

    _NC = nc
    return nc


def _prepare_in_maps(inputs, weight, grid):
    inputs = np.asarray(inputs, dtype=np.float32)
    weight = np.asarray(weight, dtype=np.float32)
    grid = np.asarray(grid, dtype=np.float32)

    gpad = np.zeros((NPAD, 2), dtype=np.float32)
    gpad[:N] = grid
    G2 = (gpad * gpad).sum(-1)
    gaug_full = np.stack([-2.0 * gpad[:, 0], -2.0 * gpad[:, 1], G2,
                          np.ones(NPAD, np.float32)], 0).astype(np.float16)

    in_maps = []
    wB = np.empty(B, np.float32)
    for b in range(B):
        wB[b] = M0 * weight[b].sum(dtype=np.float32)
    for c in range(8):
        b = c // NSHARDS
        s = c % NSHARDS
        # sort points by weight: prefixes stay spatially random (counts are
        # unweighted) while 256-col blocks get nearly-constant weight
        perm = np.argsort(weight[b], kind='stable')
        X = inputs[b][perm]
        wp16 = weight[b][perm].astype(np.float16)
        X2 = (X * X).sum(-1)
        xaug_np = np.stack([X[:, 0], X[:, 1], np.ones(M, np.float32), X2],
                           0).astype(np.float16)
        sw16 = wp16.astype(np.float32).sum(dtype=np.float32)
        in_maps.append({
            "gaug": np.ascontiguousarray(gaug_full[:, s * NSH:(s + 1) * NSH]),
            "xaug": xaug_np,
            "wrow": wp16[None, :],
            "bnd2": np.array([[M0 * sw16 - sw16]], dtype=np.float32),
        })
    return in_maps, wB


def _gather(results, wB):
    sel = np.empty((B, NPAD), np.float32)
    for c in range(8):
        b = c // NSHARDS
        s = c % NSHARDS
        vals = results[c]["out"]            # [P, NT]; grid idx = t*P + p
        sel[b, s * NSH:(s + 1) * NSH] = vals.T.reshape(-1)
    sel = sel[:, :N]
    out = np.sqrt(np.maximum(sel, 0.0) / wB[:, None]).astype(np.float32)
    return out


def _make_runner(nc, n_cores=8):
    """Compile once; return a reusable sharded callable (avoids per-call
    retracing in run_bass_kernel_spmd)."""
    import jax
    from jax.sharding import Mesh, PartitionSpec
    from jax.experimental.shard_map import shard_map
    from concourse import bass2jax
    import concourse.mybir as _mybir

    bass2jax.install_neuronx_cc_hook()
    in_names, out_names, out_avals = [], [], []
    for alloc in nc.m.functions[0].allocations:
        if not isinstance(alloc, _mybir.MemoryLocationSet):
            continue
        name = alloc.memorylocations[0].name
        if alloc.kind == "ExternalInput":
            if not (nc.partition_id_tensor is not None
                    and name == nc.partition_id_tensor.name):
                in_names.append(name)
        elif alloc.kind == "ExternalOutput":
            out_names.append(name)
            out_avals.append(jax.core.ShapedArray(
                tuple(alloc.tensor_shape), _mybir.dt.np(alloc.dtype)))
    n_params = len(in_names)
    all_names = list(in_names) + list(out_names)
    has_pid = nc.partition_id_tensor is not None
    if has_pid:
        all_names.append(nc.partition_id_tensor.name)

    def _body(*args):
        operands = list(args)
        if has_pid:
            operands.append(bass2jax.partition_id_tensor())
        outs = bass2jax._bass_exec_p.bind(
            *operands, out_avals=tuple(out_avals), in_names=tuple(all_names),
            out_names=tuple(out_names), lowering_input_output_aliases=(),
            sim_require_finite=True, sim_require_nnan=True, nc=nc)
        return tuple(outs)

    devices = jax.devices()[:n_cores]
    mesh = Mesh(np.asarray(devices), ("core",))
    nio = n_params + len(out_names)
    sharded = jax.jit(
        shard_map(_body, mesh=mesh, in_specs=(PartitionSpec("core"),) * nio,
                  out_specs=(PartitionSpec("core"),) * len(out_names),
                  check_rep=False),
        keep_unused=True)

    def run(in_maps):
        per_core = [[np.asarray(m[name]) for name in in_names] for m in in_maps]
        concat_in = [np.concatenate([per_core[c][i] for c in range(n_cores)], 0)
                     for i in range(n_params)]
        concat_zeros = [np.zeros((n_cores * a.shape[0], *a.shape[1:]), a.dtype)
                        for a in out_avals]
        outs = sharded(*concat_in, *concat_zeros)
        outs = [np.asarray(o) for o in outs]
        return [{name: outs[i].reshape(n_cores, *out_avals[i].shape)[c]
                 for i, name in enumerate(out_names)} for c in range(n_cores)]

    return run


_RUNNER = None


def _get_runner():
    global _RUNNER
    if _RUNNER is None:
        _RUNNER = _make_runner(_build())
    return _RUNNER


def kernel(inputs, weight, grid):
    in_maps, wB = _prepare_in_maps(inputs, weight, grid)
    global _RUNNER
    try:
        results = _get_runner()(in_maps)
    except Exception:
        # transient NRT/axon failures: rebuild the executable once and retry
        _RUNNER = None
        results = _get_runner()(in_maps)
    return _gather(results, wB)
